# revision 20
# baseline (speedup 1.0000x reference)
"""Trainium2 Bass kernel for AtacformerPairwiseInteractionHead.

Reference math (B=4, N=256, D=512):
    h[b,i,j,:] = relu(e_i @ Wa + e_j @ Wb + (e_i*e_j) @ Wc + |e_i-e_j| @ Wd + b1)
    scores[b,i,j] = h[b,i,j,:] @ W2 + b2

Sharding: (batch, row-block) across 8 cores — core c handles batch c//2,
i-rows [128*(c%2), 128*(c%2)+128). No collectives; each core computes a
disjoint output slice.

Per-core layout (transposed: channels on partitions, tokens j in free dim):
    E^T  (4dt, 128, 256j)  bf16   X1 = E^T * e_i, X2 = |E^T - e_i|  (DVE)
    h^T[kt] = C^T[kt] + Wc[dt,kt]^T X1 + Wd[dt,kt]^T X2   (PE, psum f32)
    h = relu(h^T + a_i[kt] + b1[kt])  (ACT, per-partition bias)  -> bf16
    scores = W2^T h  (PE, M=1)  -> psum (1, 512) = two rows of 256
Rows are processed in pairs so matmuls stream N=512.
"""

import numpy as np
import ml_dtypes

import concourse.bass as bass
import concourse.bacc as bacc
import concourse.tile as tile
import concourse.mybir as mybir
from concourse.bass_utils import run_bass_kernel_spmd

BF16 = mybir.dt.bfloat16
F32 = mybir.dt.float32
nbf16 = ml_dtypes.bfloat16

B, N, D = 4, 256, 512
N_CORES = 8
ROWS_PER_CORE = (B * N) // N_CORES  # 128


def build(n_pairs=ROWS_PER_CORE // 2, rows_override=None):
    nc = bacc.Bacc("TRN2", target_bir_lowering=False, debug=False)

    # Host-prepared per-core inputs (partition dim first).
    et_d = nc.dram_tensor("et", [128, 4, 256], BF16, kind="ExternalInput")
    eit_d = nc.dram_tensor("eit", [128, 4, 128], BF16, kind="ExternalInput")
    eitf_d = nc.dram_tensor("eitf", [128, 4, 136], F32, kind="ExternalInput")
    neitf_d = nc.dram_tensor("neitf", [128, 4, 136], F32, kind="ExternalInput")
    w1t_d = nc.dram_tensor("w1t", [128, 16, 512], BF16, kind="ExternalInput")
    id_d = nc.dram_tensor("ident", [128, 128], BF16, kind="ExternalInput")
    b1c_d = nc.dram_tensor("b1c", [128, 4], F32, kind="ExternalInput")
    w2c_d = nc.dram_tensor("w2c", [128, 4], BF16, kind="ExternalInput")
    out_d = nc.dram_tensor("out", [ROWS_PER_CORE // 2, 512], F32, kind="ExternalOutput")

    Abs = mybir.ActivationFunctionType.Abs
    Ident = mybir.ActivationFunctionType.Identity
    mult = mybir.AluOpType.mult
    add = mybir.AluOpType.add
    mx = mybir.AluOpType.max

    with tile.TileContext(nc) as tc:
        with (
            tc.tile_pool(name="const", bufs=1) as cp,
            tc.tile_pool(name="xp", bufs=4) as xp,
            tc.tile_pool(name="hp", bufs=4) as hp,
            tc.tile_pool(name="sp", bufs=4) as sp,
            tc.tile_pool(name="pb", bufs=6, space="PSUM") as pb,
            tc.tile_pool(name="psc", bufs=2, space="PSUM") as psc,
        ):
            warm = cp.tile([128, 512], BF16)
            nc.gpsimd.memset(warm[:], 0.0)
            pwu = pb.tile([128, 512], F32, tag="acc")
            for _ in range(20):
                nc.tensor.matmul(pwu[:], warm[:, 0:128], warm[:], start=True, stop=True)

            et = cp.tile([128, 4, 256], BF16)
            nc.sync.dma_start(et[:], et_d[:])
            eit = cp.tile([128, 4, 128], BF16)
            nc.scalar.dma_start(eit[:], eit_d[:])
            eitf = cp.tile([128, 4, 136], F32)
            nc.sync.dma_start(eitf[:], eitf_d[:])
            neitf = cp.tile([128, 4, 136], F32)
            nc.scalar.dma_start(neitf[:], neitf_d[:])
            w1 = cp.tile([128, 16, 512], BF16)
            nc.gpsimd.dma_start(w1[:, 4:8, :], w1t_d[:, 4:8, :])
            nc.gpsimd.dma_start(w1[:, 8:12, :], w1t_d[:, 8:12, :])
            nc.gpsimd.dma_start(w1[:, 12:16, :], w1t_d[:, 12:16, :])
            nc.gpsimd.dma_start(w1[:, 0:4, :], w1t_d[:, 0:4, :])
            b1s = cp.tile([128, 4], F32)
            nc.scalar.dma_start(b1s[:], b1c_d[:])
            w2s = cp.tile([128, 4], BF16)
            nc.scalar.dma_start(w2s[:], w2c_d[:])
            ident = cp.tile([128, 128], BF16)
            nc.sync.dma_start(ident[:], id_d[:])

            # C2[kt] = [C^T[kt] | C^T[kt]],  C^T = Wb^T E^T  (bf16)
            c2 = cp.tile([128, 4, 512], BF16)
            for kt in range(4):
                ks = slice(kt * 128, (kt + 1) * 128)
                pc = pb.tile([128, 512], F32, tag="acc")
                for dt_ in range(4):
                    nc.tensor.matmul(
                        pc[:, 0:256], w1[:, 4 + dt_, ks], et[:, dt_, :],
                        start=(dt_ == 0), stop=(dt_ == 3),
                    )
                nc.scalar.copy(c2[:, kt, 0:256], pc[:, 0:256])
                nc.scalar.copy(c2[:, kt, 256:512], pc[:, 0:256])

            # A'^T[kt] = Wa^T E_i^T + b1  (f32, per-row bias source)
            a_sb = cp.tile([128, 4, 136], F32)
            for kt in range(4):
                ks = slice(kt * 128, (kt + 1) * 128)
                pa = pb.tile([128, 512], F32, tag="acc")
                for dt_ in range(4):
                    nc.tensor.matmul(
                        pa[:, 0:128], w1[:, dt_, ks], eit[:, dt_, :],
                        start=(dt_ == 0), stop=(dt_ == 3),
                    )
                nc.scalar.activation(
                    a_sb[:, kt, 0:128], pa[:, 0:128], Ident, bias=b1s[:, kt : kt + 1]
                )

            for r in range(n_pairs):
                rows = (2 * r, 2 * r + 1)
                if rows_override and r in rows_override:
                    rows = rows_override[r]
                x1t, x2t = [], []
                for dt_ in range(4):
                    x1 = xp.tile([128, 2, 256], BF16, tag=f"x1{dt_}")
                    x2 = xp.tile([128, 2, 256], BF16, tag=f"x2{dt_}")
                    for s, il in enumerate(rows):
                        nc.vector.tensor_scalar(
                            x1[:, s, :], et[:, dt_, :],
                            eitf[:, dt_, il : il + 1], None, mult,
                        )
                        nc.scalar.activation(
                            x2[:, s, :], et[:, dt_, :], Abs,
                            bias=neitf[:, dt_, il : il + 1],
                        )
                    x1t.append(x1)
                    x2t.append(x2)

                hts = []
                for kt in range(4):
                    ks = slice(kt * 128, (kt + 1) * 128)
                    ph = pb.tile([128, 512], F32, tag="acc")
                    nc.tensor.matmul(
                        ph[:], ident[:], c2[:, kt, :], start=True, stop=False
                    )
                    for dt_ in range(4):
                        nc.tensor.matmul(
                            ph[:], w1[:, 8 + dt_, ks],
                            x1t[dt_][:].rearrange("p a b -> p (a b)"),
                            start=False, stop=False,
                        )
                    for dt_ in range(4):
                        nc.tensor.matmul(
                            ph[:], w1[:, 12 + dt_, ks],
                            x2t[dt_][:].rearrange("p a b -> p (a b)"),
                            start=False, stop=(dt_ == 3),
                        )
                    ht = hp.tile([128, 512], BF16, tag=f"h{kt}")
                    for s, il in enumerate(rows):
                        js = slice(s * 256, (s + 1) * 256)
                        nc.vector.tensor_scalar(
                            ht[:, js], ph[:, js],
                            a_sb[:, kt, il : il + 1], 0.0, add, mx,
                        )
                    hts.append(ht)

                ps = psc.tile([1, 512], F32, tag="ps")
                for kt in range(4):
                    nc.tensor.matmul(
                        ps[:], w2s[:, kt : kt + 1], hts[kt][:],
                        start=(kt == 0), stop=(kt == 3),
                    )
                ssb = sp.tile([1, 512], F32, tag="s")
                nc.vector.tensor_copy(ssb[:], ps[:])
                nc.sync.dma_start(out_d[r : r + 1, :], ssb[:])

    nc.compile()
    return nc


def make_in_maps(embeddings, W1, b1, W2):
    """Build the 8 per-core input dicts from full inputs."""
    emb = np.asarray(embeddings, np.float32)
    W1 = np.asarray(W1, np.float32)
    b1 = np.asarray(b1, np.float32)
    W2 = np.asarray(W2, np.float32)

    w1t = np.ascontiguousarray(
        W1.reshape(16, 128, 512).transpose(1, 0, 2)
    ).astype(nbf16)
    b1c = np.ascontiguousarray(b1.reshape(4, 128).T)
    w2c = W2[:, 0].reshape(4, 128).T.astype(nbf16)
    w2c = np.ascontiguousarray(w2c)
    ident = np.eye(128, dtype=nbf16)
    in_maps = []
    for c in range(N_CORES):
        b = c // 2
        i0 = 128 * (c % 2)
        ET = emb[b].T  # (512, 256)
        et = np.ascontiguousarray(
            ET.reshape(4, 128, 256).transpose(1, 0, 2)
        ).astype(nbf16)
        EiT = emb[b, i0 : i0 + 128].T  # (512, 128)
        eitf3 = np.ascontiguousarray(EiT.reshape(4, 128, 128).transpose(1, 0, 2))
        eit = eitf3.astype(nbf16)
        eitf = np.zeros((128, 4, 136), np.float32)
        eitf[:, :, :128] = eitf3
        in_maps.append(
            {"et": et, "eit": eit, "eitf": eitf, "neitf": -eitf, "w1t": w1t,
             "b1c": b1c, "w2c": w2c, "ident": ident}
        )
    return in_maps


_nc_cache = {}


def kernel(embeddings, W1, b1, W2, b2):
    if "nc" not in _nc_cache:
        _nc_cache["nc"] = build()
    nc = _nc_cache["nc"]

    in_maps = make_in_maps(embeddings, W1, b1, W2)
    res = run_bass_kernel_spmd(nc, in_maps, core_ids=list(range(N_CORES)))

    b2 = np.asarray(b2, np.float32)
    out = np.zeros((B, N, N), np.float32)
    for c in range(N_CORES):
        b = c // 2
        i0 = 128 * (c % 2)
        out[b, i0 : i0 + 128, :] = res.results[c]["out"].reshape(128, 256)
    out += b2[0]
    return out


# revision 26
# speedup vs baseline: 1.1084x; 1.1084x over previous
"""Trainium2 Bass kernel for AtacformerPairwiseInteractionHead.

Reference math (B=4, N=256, D=512):
    h[b,i,j,:] = relu(e_i @ Wa + e_j @ Wb + (e_i*e_j) @ Wc + |e_i-e_j| @ Wd + b1)
    scores[b,i,j] = h[b,i,j,:] @ W2 + b2

Sharding: (batch, row-block) across 8 cores — core c handles batch c//2,
i-rows [128*(c%2), 128*(c%2)+128). No collectives; each core computes a
disjoint output slice.

Per-core layout (transposed: channels on partitions, tokens j in free dim):
    E^T  (4dt, 128, 256j)  bf16
    X1 = E^T * e_i (DVE/ACT), X2 = |E^T - e_i| (ACT Abs, bias=-e_i)
    psum[kt] = sum_dt Wc[dt,kt]^T X1[dt] + Wd[dt,kt]^T X2[dt]  (PE, f32)
    hpre = psum + C^T[kt]            (DVE tensor_add, bf16)
    h = max(hpre + a_i[kt], 0)       (DVE fused add+max, per-partition bias)
    scores = W2^T h  (PE, M=1) -> psum (1, 512) = two rows of 256
Rows are processed in pairs so matmuls stream N=512 bf16 columns
(~216 ns warm); C^T = Wb^T E^T and A' = Wa^T E_i^T + b1 are precomputed
on-device. b2 is added on the host.
"""

import numpy as np
import ml_dtypes

import concourse.bass as bass
import concourse.bacc as bacc
import concourse.tile as tile
import concourse.mybir as mybir
from concourse.bass_utils import run_bass_kernel_spmd

BF16 = mybir.dt.bfloat16
F32 = mybir.dt.float32
nbf16 = ml_dtypes.bfloat16

B, N, D = 4, 256, 512
N_CORES = 8
ROWS_PER_CORE = (B * N) // N_CORES  # 128


USE_STT = False


def build(n_pairs=ROWS_PER_CORE // 2, rows_override=None, use_stt=None):
    if use_stt is None:
        use_stt = USE_STT
    nc = bacc.Bacc("TRN2", target_bir_lowering=False, debug=False)

    # Host-prepared per-core inputs (partition dim first).
    et_d = nc.dram_tensor("et", [128, 4, 256], BF16, kind="ExternalInput")
    eit_d = nc.dram_tensor("eit", [128, 4, 128], BF16, kind="ExternalInput")
    eitf_d = nc.dram_tensor("eitf", [128, 4, 136], F32, kind="ExternalInput")
    neitf_d = nc.dram_tensor("neitf", [128, 4, 136], F32, kind="ExternalInput")
    w1t_d = nc.dram_tensor("w1t", [128, 16, 512], BF16, kind="ExternalInput")
    id_d = nc.dram_tensor("ident", [128, 128], BF16, kind="ExternalInput")
    b1c_d = nc.dram_tensor("b1c", [128, 4], F32, kind="ExternalInput")
    w2c_d = nc.dram_tensor("w2c", [128, 4], BF16, kind="ExternalInput")
    out_d = nc.dram_tensor("out", [ROWS_PER_CORE // 2, 512], F32, kind="ExternalOutput")

    Abs = mybir.ActivationFunctionType.Abs
    Copy = mybir.ActivationFunctionType.Copy
    Ident = mybir.ActivationFunctionType.Identity
    mult = mybir.AluOpType.mult
    add = mybir.AluOpType.add
    mx = mybir.AluOpType.max

    with tile.TileContext(nc) as tc:
        with (
            tc.tile_pool(name="const", bufs=1) as cp,
            tc.tile_pool(name="xp", bufs=4) as xp,
            tc.tile_pool(name="hp", bufs=6) as hp,
            tc.tile_pool(name="sp", bufs=4) as sp,
            tc.tile_pool(name="pb", bufs=5, space="PSUM") as pb,
            tc.tile_pool(name="psc", bufs=3, space="PSUM") as psc,
        ):
            warm = cp.tile([128, 512], BF16)
            nc.gpsimd.memset(warm[:], 0.0)
            pwu = pb.tile([128, 512], F32, tag="acc")
            for _ in range(20):
                nc.tensor.matmul(pwu[:], warm[:, 0:128], warm[:], start=True, stop=True)

            et = cp.tile([128, 4, 256], BF16)
            nc.sync.dma_start(et[:], et_d[:])
            eit = cp.tile([128, 4, 128], BF16)
            nc.scalar.dma_start(eit[:], eit_d[:])
            eitf = cp.tile([128, 4, 136], F32)
            nc.sync.dma_start(eitf[:], eitf_d[:])
            neitf = cp.tile([128, 4, 136], F32)
            nc.scalar.dma_start(neitf[:], neitf_d[:])
            w1 = cp.tile([128, 16, 512], BF16)
            nc.gpsimd.dma_start(w1[:, 4:8, :], w1t_d[:, 4:8, :])
            nc.gpsimd.dma_start(w1[:, 8:12, :], w1t_d[:, 8:12, :])
            nc.gpsimd.dma_start(w1[:, 12:16, :], w1t_d[:, 12:16, :])
            nc.gpsimd.dma_start(w1[:, 0:4, :], w1t_d[:, 0:4, :])
            b1s = cp.tile([128, 4], F32)
            nc.scalar.dma_start(b1s[:], b1c_d[:])
            w2s = cp.tile([128, 4], BF16)
            nc.scalar.dma_start(w2s[:], w2c_d[:])
            ident = None
            if not use_stt:
                ident = cp.tile([128, 128], BF16)
                nc.sync.dma_start(ident[:], id_d[:])

            # C2[kt] = [C^T[kt] | C^T[kt]],  C^T = Wb^T E^T  (bf16)
            c2 = cp.tile([128, 4, 512], BF16)
            for kt in range(4):
                ks = slice(kt * 128, (kt + 1) * 128)
                pc = pb.tile([128, 512], F32, tag="acc")
                for dt_ in range(4):
                    nc.tensor.matmul(
                        pc[:, 0:256], w1[:, 4 + dt_, ks], et[:, dt_, :],
                        start=(dt_ == 0), stop=(dt_ == 3),
                    )
                nc.scalar.copy(c2[:, kt, 0:256], pc[:, 0:256])
                nc.scalar.copy(c2[:, kt, 256:512], pc[:, 0:256])

            # A'^T[kt] = Wa^T E_i^T + b1  (f32, per-row bias source)
            a_sb = cp.tile([128, 4, 136], F32)
            for kt in range(4):
                ks = slice(kt * 128, (kt + 1) * 128)
                pa = pb.tile([128, 512], F32, tag="acc")
                for dt_ in range(4):
                    nc.tensor.matmul(
                        pa[:, 0:128], w1[:, dt_, ks], eit[:, dt_, :],
                        start=(dt_ == 0), stop=(dt_ == 3),
                    )
                nc.scalar.activation(
                    a_sb[:, kt, 0:128], pa[:, 0:128], Ident, bias=b1s[:, kt : kt + 1]
                )

            for r in range(n_pairs):
                rows = (2 * r, 2 * r + 1)
                if rows_override and r in rows_override:
                    rows = rows_override[r]
                x1t, x2t = [], []
                for dt_ in range(4):
                    x1 = xp.tile([128, 2, 256], BF16, tag=f"x1{dt_}")
                    x2 = xp.tile([128, 2, 256], BF16, tag=f"x2{dt_}")
                    for s, il in enumerate(rows):
                        if dt_ < 2:
                            nc.scalar.activation(
                                x1[:, s, :], et[:, dt_, :], Copy,
                                scale=eitf[:, dt_, il : il + 1],
                            )
                        else:
                            nc.vector.tensor_scalar(
                                x1[:, s, :], et[:, dt_, :],
                                eitf[:, dt_, il : il + 1], None, mult,
                            )
                        nc.scalar.activation(
                            x2[:, s, :], et[:, dt_, :], Abs,
                            bias=neitf[:, dt_, il : il + 1],
                        )
                    x1t.append(x1)
                    x2t.append(x2)

                hts = []
                for kt in range(4):
                    ks = slice(kt * 128, (kt + 1) * 128)
                    ph = pb.tile([128, 512], F32, tag="acc")
                    if not use_stt:
                        nc.tensor.matmul(
                            ph[:], ident[:], c2[:, kt, :], start=True, stop=False
                        )
                    for dt_ in range(4):
                        nc.tensor.matmul(
                            ph[:], w1[:, 8 + dt_, ks],
                            x1t[dt_][:].rearrange("p a b -> p (a b)"),
                            start=(use_stt and dt_ == 0), stop=False,
                        )
                    for dt_ in range(4):
                        nc.tensor.matmul(
                            ph[:], w1[:, 12 + dt_, ks],
                            x2t[dt_][:].rearrange("p a b -> p (a b)"),
                            start=False, stop=(dt_ == 3),
                        )
                    ht = hp.tile([128, 512], BF16, tag=f"h{kt}")
                    if use_stt:
                        hpre = hp.tile([128, 512], BF16, tag=f"hp{kt}")
                        nc.vector.tensor_add(hpre[:], ph[:], c2[:, kt, :])
                        for s, il in enumerate(rows):
                            js = slice(s * 256, (s + 1) * 256)
                            nc.vector.tensor_scalar(
                                ht[:, js], hpre[:, js],
                                a_sb[:, kt, il : il + 1], 0.0, add, mx,
                            )
                    else:
                        for s, il in enumerate(rows):
                            js = slice(s * 256, (s + 1) * 256)
                            nc.vector.tensor_scalar(
                                ht[:, js], ph[:, js],
                                a_sb[:, kt, il : il + 1], 0.0, add, mx,
                            )
                    hts.append(ht)

                ps = psc.tile([1, 512], F32, tag="ps")
                for kt in range(4):
                    nc.tensor.matmul(
                        ps[:], w2s[:, kt : kt + 1], hts[kt][:],
                        start=(kt == 0), stop=(kt == 3),
                    )
                ssb = sp.tile([1, 512], F32, tag="s")
                nc.scalar.copy(ssb[:], ps[:])
                nc.sync.dma_start(out_d[r : r + 1, :], ssb[:])

    nc.compile()
    return nc


def make_in_maps(embeddings, W1, b1, W2):
    """Build the 8 per-core input dicts from full inputs."""
    emb = np.asarray(embeddings, np.float32)
    W1 = np.asarray(W1, np.float32)
    b1 = np.asarray(b1, np.float32)
    W2 = np.asarray(W2, np.float32)

    w1t = np.ascontiguousarray(
        W1.reshape(16, 128, 512).transpose(1, 0, 2)
    ).astype(nbf16)
    b1c = np.ascontiguousarray(b1.reshape(4, 128).T)
    w2c = W2[:, 0].reshape(4, 128).T.astype(nbf16)
    w2c = np.ascontiguousarray(w2c)
    ident = np.eye(128, dtype=nbf16)
    in_maps = []
    for c in range(N_CORES):
        b = c // 2
        i0 = 128 * (c % 2)
        ET = emb[b].T  # (512, 256)
        et = np.ascontiguousarray(
            ET.reshape(4, 128, 256).transpose(1, 0, 2)
        ).astype(nbf16)
        EiT = emb[b, i0 : i0 + 128].T  # (512, 128)
        eitf3 = np.ascontiguousarray(EiT.reshape(4, 128, 128).transpose(1, 0, 2))
        eit = eitf3.astype(nbf16)
        eitf = np.zeros((128, 4, 136), np.float32)
        eitf[:, :, :128] = eitf3
        in_maps.append(
            {"et": et, "eit": eit, "eitf": eitf, "neitf": -eitf, "w1t": w1t,
             "b1c": b1c, "w2c": w2c, "ident": ident}
        )
    return in_maps


_nc_cache = {}


def kernel(embeddings, W1, b1, W2, b2):
    if "nc" not in _nc_cache:
        _nc_cache["nc"] = build()
    nc = _nc_cache["nc"]

    in_maps = make_in_maps(embeddings, W1, b1, W2)
    res = run_bass_kernel_spmd(nc, in_maps, core_ids=list(range(N_CORES)))

    b2 = np.asarray(b2, np.float32)
    out = np.zeros((B, N, N), np.float32)
    for c in range(N_CORES):
        b = c // 2
        i0 = 128 * (c % 2)
        out[b, i0 : i0 + 128, :] = res.results[c]["out"].reshape(128, 256)
    out += b2[0]
    return out


# revision 29
# speedup vs baseline: 1.3617x; 1.2286x over previous
"""Trainium2 Bass kernel for AtacformerPairwiseInteractionHead.

Reference math (B=4, N=256, D=512):
    h[b,i,j,:] = relu(e_i @ Wa + e_j @ Wb + (e_i*e_j) @ Wc + |e_i-e_j| @ Wd + b1)
    scores[b,i,j] = h[b,i,j,:] @ W2 + b2

Sharding: (batch, row-block) across 8 cores — core c handles batch c//2,
i-rows [128*(c%2), 128*(c%2)+128). No collectives; each core computes a
disjoint output slice.

Per-core layout (transposed: channels on partitions, tokens j in free dim):
    E^T  (4dt, 128, 256j)  bf16
    X1 = E^T * e_i (DVE/ACT), X2 = |E^T - e_i| (ACT Abs, bias=-e_i)
    psum[kt] = sum_dt Wc[dt,kt]^T X1[dt] + Wd[dt,kt]^T X2[dt]  (PE, f32)
    hpre = psum + C^T[kt]            (DVE tensor_add, bf16)
    h = max(hpre + a_i[kt], 0)       (DVE fused add+max, per-partition bias)
    scores = W2^T h  (PE, M=1) -> psum (1, 512) = two rows of 256
Rows are processed in pairs so matmuls stream N=512 bf16 columns
(~216 ns warm); C^T = Wb^T E^T and A' = Wa^T E_i^T + b1 are precomputed
on-device. b2 is added on the host.
"""

import numpy as np
import ml_dtypes

import concourse.bass as bass
import concourse.bacc as bacc
import concourse.tile as tile
import concourse.mybir as mybir
from concourse.bass_utils import run_bass_kernel_spmd

BF16 = mybir.dt.bfloat16
F32 = mybir.dt.float32
nbf16 = ml_dtypes.bfloat16

B, N, D = 4, 256, 512
N_CORES = 8
ROWS_PER_CORE = (B * N) // N_CORES  # 128


USE_STT = False


def build(n_pairs=ROWS_PER_CORE // 2, rows_override=None, use_stt=None):
    if use_stt is None:
        use_stt = USE_STT
    nc = bacc.Bacc("TRN2", target_bir_lowering=False, debug=False)

    # Host-prepared per-core inputs (partition dim first).
    et_d = nc.dram_tensor("et", [128, 4, 256], BF16, kind="ExternalInput")
    eit_d = nc.dram_tensor("eit", [128, 4, 128], BF16, kind="ExternalInput")
    eitf_d = nc.dram_tensor("eitf", [128, 4, 136], F32, kind="ExternalInput")
    neitf_d = nc.dram_tensor("neitf", [128, 4, 136], F32, kind="ExternalInput")
    w1t_d = nc.dram_tensor("w1t", [128, 16, 512], BF16, kind="ExternalInput")
    id_d = nc.dram_tensor("ident", [128, 128], BF16, kind="ExternalInput")
    b1c_d = nc.dram_tensor("b1c", [128, 4], F32, kind="ExternalInput")
    w2c_d = nc.dram_tensor("w2c", [128, 4], BF16, kind="ExternalInput")
    out_d = nc.dram_tensor("out", [ROWS_PER_CORE // 2, 512], F32, kind="ExternalOutput")

    Abs = mybir.ActivationFunctionType.Abs
    Copy = mybir.ActivationFunctionType.Copy
    Ident = mybir.ActivationFunctionType.Identity
    mult = mybir.AluOpType.mult
    add = mybir.AluOpType.add
    mx = mybir.AluOpType.max

    with tile.TileContext(nc) as tc:
        with (
            tc.tile_pool(name="const", bufs=1) as cp,
            tc.tile_pool(name="xp", bufs=6) as xp,
            tc.tile_pool(name="hp", bufs=6) as hp,
            tc.tile_pool(name="sp", bufs=4) as sp,
            tc.tile_pool(name="pb", bufs=6, space="PSUM") as pb,
            tc.tile_pool(name="psc", bufs=2, space="PSUM") as psc,
        ):
            warm = cp.tile([128, 512], BF16)
            nc.vector.memset(warm[:], 0.0)
            pwu = pb.tile([128, 512], F32, tag="acc")
            for _ in range(10):
                nc.tensor.matmul(pwu[:], warm[:, 0:128], warm[:], start=True, stop=True)

            et = cp.tile([128, 4, 256], BF16)
            nc.sync.dma_start(et[:], et_d[:])
            eit = cp.tile([128, 4, 128], BF16)
            nc.scalar.dma_start(eit[:], eit_d[:])
            eitf = cp.tile([128, 4, 136], F32)
            nc.sync.dma_start(eitf[:], eitf_d[:])
            neitf = cp.tile([128, 4, 136], F32)
            nc.scalar.dma_start(neitf[:], neitf_d[:])
            w1 = cp.tile([128, 16, 512], BF16)
            nc.gpsimd.dma_start(w1[:, 4:8, :], w1t_d[:, 4:8, :])
            nc.gpsimd.dma_start(w1[:, 8:12, :], w1t_d[:, 8:12, :])
            nc.gpsimd.dma_start(w1[:, 12:16, :], w1t_d[:, 12:16, :])
            nc.gpsimd.dma_start(w1[:, 0:4, :], w1t_d[:, 0:4, :])
            b1s = cp.tile([128, 4], F32)
            nc.scalar.dma_start(b1s[:], b1c_d[:])
            w2s = cp.tile([128, 4], BF16)
            nc.scalar.dma_start(w2s[:], w2c_d[:])
            ident = None
            if not use_stt:
                ident = cp.tile([128, 128], BF16)
                nc.sync.dma_start(ident[:], id_d[:])

            # C2[kt] = [C^T[kt] | C^T[kt]],  C^T = Wb^T E^T  (bf16)
            c2 = cp.tile([128, 4, 512], BF16)
            for kt in range(4):
                ks = slice(kt * 128, (kt + 1) * 128)
                pc = pb.tile([128, 512], F32, tag="acc")
                for dt_ in range(4):
                    nc.tensor.matmul(
                        pc[:, 0:256], w1[:, 4 + dt_, ks], et[:, dt_, :],
                        start=(dt_ == 0), stop=(dt_ == 3),
                    )
                nc.scalar.copy(c2[:, kt, 0:256], pc[:, 0:256])
                nc.scalar.copy(c2[:, kt, 256:512], pc[:, 0:256])

            # A'^T[kt] = Wa^T E_i^T + b1  (f32, per-row bias source)
            a_sb = cp.tile([128, 4, 136], F32)
            for kt in range(4):
                ks = slice(kt * 128, (kt + 1) * 128)
                pa = pb.tile([128, 512], F32, tag="acc")
                for dt_ in range(4):
                    nc.tensor.matmul(
                        pa[:, 0:128], w1[:, dt_, ks], eit[:, dt_, :],
                        start=(dt_ == 0), stop=(dt_ == 3),
                    )
                nc.scalar.activation(
                    a_sb[:, kt, 0:128], pa[:, 0:128], Ident, bias=b1s[:, kt : kt + 1]
                )

            def emit_w2(r, hts):
                ps = psc.tile([1, 512], F32, tag="ps")
                for kt in range(4):
                    nc.tensor.matmul(
                        ps[:], w2s[:, kt : kt + 1], hts[kt][:],
                        start=(kt == 0), stop=(kt == 3),
                    )
                ssb = sp.tile([1, 512], F32, tag="s")
                nc.scalar.copy(ssb[:], ps[:])
                nc.sync.dma_start(out_d[r : r + 1, :], ssb[:])

            pending = None
            for r in range(n_pairs):
                rows = (2 * r, 2 * r + 1)
                if rows_override and r in rows_override:
                    rows = rows_override[r]
                x1t, x2t = [], []
                for dt_ in range(4):
                    x1 = xp.tile([128, 2, 256], BF16, tag=f"x1{dt_}")
                    x2 = xp.tile([128, 2, 256], BF16, tag=f"x2{dt_}")
                    for s, il in enumerate(rows):
                        if dt_ < 2:
                            nc.scalar.activation(
                                x1[:, s, :], et[:, dt_, :], Copy,
                                scale=eitf[:, dt_, il : il + 1],
                            )
                        else:
                            nc.vector.tensor_scalar(
                                x1[:, s, :], et[:, dt_, :],
                                eitf[:, dt_, il : il + 1], None, mult,
                            )
                        nc.scalar.activation(
                            x2[:, s, :], et[:, dt_, :], Abs,
                            bias=neitf[:, dt_, il : il + 1],
                        )
                    x1t.append(x1)
                    x2t.append(x2)

                hts = []
                for kt in range(4):
                    ks = slice(kt * 128, (kt + 1) * 128)
                    ph = pb.tile([128, 512], F32, tag="acc")
                    if not use_stt:
                        nc.tensor.matmul(
                            ph[:], ident[:], c2[:, kt, :], start=True, stop=False
                        )
                    for dt_ in range(4):
                        nc.tensor.matmul(
                            ph[:], w1[:, 8 + dt_, ks],
                            x1t[dt_][:].rearrange("p a b -> p (a b)"),
                            start=(use_stt and dt_ == 0), stop=False,
                        )
                    for dt_ in range(4):
                        nc.tensor.matmul(
                            ph[:], w1[:, 12 + dt_, ks],
                            x2t[dt_][:].rearrange("p a b -> p (a b)"),
                            start=False, stop=(dt_ == 3),
                        )
                    ht = hp.tile([128, 512], BF16, tag=f"h{kt}")
                    if use_stt:
                        hpre = hp.tile([128, 512], BF16, tag=f"hp{kt}")
                        nc.vector.tensor_add(hpre[:], ph[:], c2[:, kt, :])
                        for s, il in enumerate(rows):
                            js = slice(s * 256, (s + 1) * 256)
                            nc.vector.tensor_scalar(
                                ht[:, js], hpre[:, js],
                                a_sb[:, kt, il : il + 1], 0.0, add, mx,
                            )
                    else:
                        for s, il in enumerate(rows):
                            js = slice(s * 256, (s + 1) * 256)
                            nc.vector.tensor_scalar(
                                ht[:, js], ph[:, js],
                                a_sb[:, kt, il : il + 1], 0.0, add, mx,
                            )
                    hts.append(ht)

                if pending is not None:
                    emit_w2(*pending)
                pending = (r, hts)
            if pending is not None:
                emit_w2(*pending)

    nc.compile()
    return nc


def make_in_maps(embeddings, W1, b1, W2):
    """Build the 8 per-core input dicts from full inputs."""
    emb = np.asarray(embeddings, np.float32)
    W1 = np.asarray(W1, np.float32)
    b1 = np.asarray(b1, np.float32)
    W2 = np.asarray(W2, np.float32)

    w1t = np.ascontiguousarray(
        W1.reshape(16, 128, 512).transpose(1, 0, 2)
    ).astype(nbf16)
    b1c = np.ascontiguousarray(b1.reshape(4, 128).T)
    w2c = W2[:, 0].reshape(4, 128).T.astype(nbf16)
    w2c = np.ascontiguousarray(w2c)
    ident = np.eye(128, dtype=nbf16)
    in_maps = []
    for c in range(N_CORES):
        b = c // 2
        i0 = 128 * (c % 2)
        ET = emb[b].T  # (512, 256)
        et = np.ascontiguousarray(
            ET.reshape(4, 128, 256).transpose(1, 0, 2)
        ).astype(nbf16)
        EiT = emb[b, i0 : i0 + 128].T  # (512, 128)
        eitf3 = np.ascontiguousarray(EiT.reshape(4, 128, 128).transpose(1, 0, 2))
        eit = eitf3.astype(nbf16)
        eitf = np.zeros((128, 4, 136), np.float32)
        eitf[:, :, :128] = eitf3
        in_maps.append(
            {"et": et, "eit": eit, "eitf": eitf, "neitf": -eitf, "w1t": w1t,
             "b1c": b1c, "w2c": w2c, "ident": ident}
        )
    return in_maps


_nc_cache = {}


def kernel(embeddings, W1, b1, W2, b2):
    if "nc" not in _nc_cache:
        _nc_cache["nc"] = build()
    nc = _nc_cache["nc"]

    in_maps = make_in_maps(embeddings, W1, b1, W2)
    res = run_bass_kernel_spmd(nc, in_maps, core_ids=list(range(N_CORES)))

    b2 = np.asarray(b2, np.float32)
    out = np.zeros((B, N, N), np.float32)
    for c in range(N_CORES):
        b = c // 2
        i0 = 128 * (c % 2)
        out[b, i0 : i0 + 128, :] = res.results[c]["out"].reshape(128, 256)
    out += b2[0]
    return out
